# revision 1
# baseline (speedup 1.0000x reference)
"""Trainium2 Bass kernel for nn_Block_39779987095924 (GSPN-style block), v2.

Sharding: 8 cores = 4 images x 2 channel-blocks (cb). Head (LN) duplicated
per pair; cross-core joins via pairwise DRAM collectives:
  xdown partial -> AllReduce; merged -> AllGather; y2 -> AllGather.
Per-core: in_proj (my 128 ch) -> dwconv7 (rows split DVE/Pool/PE-diag) ->
xdown partial -> gates/L/U/D (4-direction row-tiled PE matmuls, Act evac)
-> bf16 chunk-2 scan algebra -> outconv (my 128 out-ch) -> dwconv3 ->
relu^2 -> outproj (my 128 out cols) -> out [128, T] (host transposes).
"""

import sys

sys.path.insert(0, "/opt/trn_rl_repo")

from contextlib import ExitStack

import numpy as np
import ml_dtypes

import concourse.bass as bass
import concourse.bacc as bacc
import concourse.tile as tile
from concourse import mybir
from concourse.bass_utils import run_bass_kernel_spmd
from concourse.masks import make_identity

B, T, D = 4, 4096, 256
HW = 64
DS = 16
EPS = 1e-5
NCORES = 8
PAIRS = [[0, 1], [2, 3], [4, 5], [6, 7]]

F32 = mybir.dt.float32
BF16 = mybir.dt.bfloat16

Alu = mybir.AluOpType
Act = mybir.ActivationFunctionType

TAPS7 = [(0, 0)] + [
    (di, dj) for di in range(-3, 4) for dj in range(-3, 4) if (di, dj) != (0, 0)
]
TAPS3 = [(0, 0)] + [
    (di, dj) for di in range(-1, 2) for dj in range(-1, 2) if (di, dj) != (0, 0)
]

# conv7 row split: PE range must be 16-aligned (2-bank psum blocks)
C7_DVE = (0, 16)
C7_PE = (16, 64)
# dwconv3 row split
D3_DVE = (0, 24)
D3_PE = (24, 64)


def _bf(x):
    return np.ascontiguousarray(np.asarray(x).astype(ml_dtypes.bfloat16))


def _f32(x):
    return np.ascontiguousarray(np.asarray(x, dtype=np.float32))


def host_prep(inputs):
    hs = _f32(inputs["hidden_states"])
    w_in = _f32(inputs["in_proj_w"])
    gamma = _f32(inputs["norm_w"])
    beta = _f32(inputs["norm_b"])
    conv7_w = _f32(inputs["conv7_w"])[:, 0]
    conv7_b = _f32(inputs["conv7_b"])
    xdown_w = _f32(inputs["xdown_w"])
    wup_w = _f32(inputs["wup_w"])
    lup_w = _f32(inputs["lup_w"])
    uup_w = _f32(inputs["uup_w"])
    dup_w = _f32(inputs["dup_w"])
    m_w = _f32(inputs["m_w"])
    outconv_w = _f32(inputs["outconv_w"])
    outdconv_w = _f32(inputs["outdconv_w"])[:, 0]
    outproj_w = _f32(inputs["outproj_w"])

    w_eff = (w_in * gamma[None, :]).T
    in_bias = w_in @ beta

    k7 = np.stack([conv7_w[:, 3 + di, 3 + dj] for (di, dj) in TAPS7], 1)
    k3 = np.stack([outdconv_w[:, 1 + di, 1 + dj] for (di, dj) in TAPS3], 1)

    eye = np.eye(128, dtype=np.float32)
    in_maps = []
    for core in range(NCORES):
        b, cb = core // 2, core % 2
        ch = slice(cb * 128, cb * 128 + 128)

        gw = np.zeros((128, 6, 128), np.float32)
        for f in range(4):
            blocks = [
                wup_w[f * D:(f + 1) * D][ch],
                wup_w[4 * D + f * D:4 * D + (f + 1) * D][ch],
                wup_w[8 * D + f * D:8 * D + (f + 1) * D][ch],
                lup_w[f * D:(f + 1) * D][ch],
                uup_w[f * D:(f + 1) * D][ch] * m_w[f],
                dup_w[f * D:(f + 1) * D][ch] * m_w[f],
            ]
            for q, blk in enumerate(blocks):
                gw[32 * f:32 * f + 16, q, :] = blk.T

        # diag7[p, t, m] = k7[ch][p, t] * (p == m)
        diag7 = eye[:, None, :] * k7[ch][:, :, None]
        k3c = k3[ch]
        diag3 = eye[:, None, :] * k3c[:, :, None]

        m = {
            "hs": hs[b],
            "winT": _bf(w_eff[:, ch]),
            "inb": _f32(in_bias[ch].reshape(128, 1)),
            "k7": _f32(k7[ch]),
            "c7b": _f32(conv7_b[ch].reshape(128, 1)),
            "diag7": _bf(diag7),
            "xdT": _bf(xdown_w[:, ch].T),
            "gw": _bf(gw),
            "k3": _f32(k3c),
            "diag3": _bf(diag3),
            "ocT": _bf(outconv_w.T[:, ch].reshape(2, 128, 128)),
            "opT": _bf(outproj_w.T[:, ch].reshape(2, 128, 128)),
        }
        in_maps.append(m)
    return in_maps


def build_program():
    nc = bacc.Bacc(num_devices=NCORES)
    hs_d = nc.dram_tensor("hs", [T, D], F32, kind="ExternalInput")
    winT_d = nc.dram_tensor("winT", [D, 128], BF16, kind="ExternalInput")
    inb_d = nc.dram_tensor("inb", [128, 1], F32, kind="ExternalInput")
    k7_d = nc.dram_tensor("k7", [128, 49], F32, kind="ExternalInput")
    c7b_d = nc.dram_tensor("c7b", [128, 1], F32, kind="ExternalInput")
    diag7_d = nc.dram_tensor("diag7", [128, 49, 128], BF16, kind="ExternalInput")
    xdT_d = nc.dram_tensor("xdT", [128, DS], BF16, kind="ExternalInput")
    gw_d = nc.dram_tensor("gw", [128, 6, 128], BF16, kind="ExternalInput")
    k3_d = nc.dram_tensor("k3", [128, 9], F32, kind="ExternalInput")
    diag3_d = nc.dram_tensor("diag3", [128, 9, 128], BF16, kind="ExternalInput")
    ocT_d = nc.dram_tensor("ocT", [2, 128, 128], BF16, kind="ExternalInput")
    opT_d = nc.dram_tensor("opT", [2, 128, 128], BF16, kind="ExternalInput")
    out_d = nc.dram_tensor("out", [128, T], F32, kind="ExternalOutput")

    with tile.TileContext(nc) as tc, ExitStack() as ctx:
        const = ctx.enter_context(tc.tile_pool(name="const", bufs=1))
        big = ctx.enter_context(tc.tile_pool(name="big", bufs=1))
        mmo = ctx.enter_context(tc.tile_pool(name="mmo", bufs=2))
        lud = ctx.enter_context(tc.tile_pool(name="lud", bufs=1))
        tr = ctx.enter_context(tc.tile_pool(name="tr", bufs=1))
        st = ctx.enter_context(tc.tile_pool(name="st", bufs=4))
        rl = ctx.enter_context(tc.tile_pool(name="rl", bufs=1))
        ps2 = ctx.enter_context(tc.tile_pool(name="ps2", bufs=3, space="PSUM"))
        pst = ctx.enter_context(tc.tile_pool(name="pst", bufs=1, space="PSUM"))
        dram = ctx.enter_context(tc.tile_pool(name="dram", bufs=1, space="DRAM"))

        hT = big.tile([128, 2, T], BF16, tag="hT")
        hs_half = [big.tile([128, 16, D], F32, tag=f"hs{i}", name=f"hsh{i}")
                   for i in range(2)]
        hs_v = hs_d.rearrange("(n p) d -> p n d", p=128)
        for q in range(8):
            nc.sync.dma_start(out=hs_half[q // 4][:, (q % 4) * 4:(q % 4) * 4 + 4, :],
                              in_=hs_v[:, q * 4:(q + 1) * 4, :])

        # ---- constants ----
        winT_sb = const.tile([128, 2, 128], BF16)
        nc.sync.dma_start(out=winT_sb, in_=winT_d.rearrange("(a p) m -> p a m", p=128))
        inb_sb = const.tile([128, 1], F32)
        nc.sync.dma_start(out=inb_sb, in_=inb_d[:, :])
        k7_sb = const.tile([128, 49], F32)
        nc.sync.dma_start(out=k7_sb, in_=k7_d[:, :])
        c7b_sb = const.tile([128, 1], F32)
        nc.sync.dma_start(out=c7b_sb, in_=c7b_d[:, :])
        diag7_sb = const.tile([128, 49, 128], BF16)
        nc.sync.dma_start(out=diag7_sb, in_=diag7_d[:, :, :])
        xdT_sb = const.tile([128, DS], BF16)
        nc.sync.dma_start(out=xdT_sb, in_=xdT_d[:, :])
        gw_sb = const.tile([128, 6, 128], BF16)
        nc.sync.dma_start(out=gw_sb, in_=gw_d[:, :, :])
        k3_sb = const.tile([128, 9], F32)
        nc.sync.dma_start(out=k3_sb, in_=k3_d[:, :])
        diag3_sb = const.tile([128, 9, 128], BF16)
        nc.sync.dma_start(out=diag3_sb, in_=diag3_d[:, :, :])
        ocT_sb = const.tile([128, 2, 128], BF16)
        nc.sync.dma_start(out=ocT_sb, in_=ocT_d.rearrange("a p m -> p a m"))
        opT_sb = const.tile([128, 2, 128], BF16)
        nc.sync.dma_start(out=opT_sb, in_=opT_d.rearrange("a p m -> p a m"))
        eps_sb = st.tile([128, 1], F32, tag="eps")
        nc.vector.memset(eps_sb, EPS)
        ident = const.tile([128, 128], BF16, tag="ident")
        make_identity(nc, ident)

        # ---- LN + transpose -> hT [128, 2, T] bf16 ----

        for g in range(8):
            ptr = pst.tile([128, 2, 512], BF16, tag="ptr")
            for ti in range(4):
                tt = g * 4 + ti
                xs_t = hs_half[tt // 16][:, tt % 16, :]
                stat = st.tile([128, 6], F32, tag="stat")
                nc.vector.bn_stats(out=stat, in_=xs_t)
                mv = st.tile([128, 2], F32, tag="mv")
                nc.vector.bn_aggr(out=mv, in_=stat)
                rstd = st.tile([128, 1], F32, tag="rstd")
                nc.scalar.activation(out=rstd, in_=mv[:, 1:2], func=Act.Sqrt,
                                     bias=eps_sb, scale=1.0)
                nc.vector.reciprocal(out=rstd, in_=rstd)
                h_bf = st.tile([128, D], BF16, tag="hbf")
                nc.vector.tensor_scalar(out=h_bf, in0=xs_t, scalar1=mv[:, 0:1],
                                        scalar2=rstd, op0=Alu.subtract, op1=Alu.mult)
                for kb in range(2):
                    nc.tensor.transpose(ptr[:, kb, ti * 128:(ti + 1) * 128],
                                        h_bf[:, kb * 128:(kb + 1) * 128], ident)
            for kb in range(2):
                nc.scalar.copy(out=hT[:, kb, g * 512:(g + 1) * 512], in_=ptr[:, kb, :])

        # ---- in_proj -> vpad [128, 70, 70] bf16 (zero-padded) ----
        vpad = big.tile([128, 70, 70], BF16, tag="vpad")
        nc.gpsimd.memset(vpad, 0.0)
        for c4 in range(4):          # 16 rows per fill
            pt = ps2.tile([128, 1024], F32, tag="ps2")
            for h in range(2):
                tb = c4 * 2 + h
                for kb in range(2):
                    nc.tensor.matmul(pt[:, h * 512:(h + 1) * 512], winT_sb[:, kb, :],
                                     hT[:, kb, tb * 512:(tb + 1) * 512],
                                     start=(kb == 0), stop=(kb == 1))
            nc.scalar.activation(
                out=vpad[:, 3 + c4 * 16:3 + c4 * 16 + 16, 3:67],
                in_=pt, func=Act.Identity, bias=inb_sb, scale=1.0)

        # ---- dwconv7: rows split across DVE / Pool / PE ----
        xc = big.tile([128, HW, HW], BF16, tag="xc")
        r0, r1 = C7_DVE
        acc = big.tile([128, 32, HW], F32, tag="accd", name="acc7")[:, 0:r1 - r0, :]
        nc.vector.tensor_scalar(out=acc, in0=vpad[:, 3 + r0:3 + r1, 3:67],
                                scalar1=k7_sb[:, 0:1], scalar2=c7b_sb,
                                op0=Alu.mult, op1=Alu.add)
        for t, (di, dj) in enumerate(TAPS7):
            if t == 0:
                continue
            srcv = vpad[:, 3 + r0 + di:3 + r1 + di, 3 + dj:67 + dj]
            nc.vector.scalar_tensor_tensor(out=acc, in0=srcv,
                                           scalar=k7_sb[:, t:t + 1], in1=acc,
                                           op0=Alu.mult, op1=Alu.add)
        nc.vector.tensor_copy(out=xc[:, r0:r1, :], in_=acc)
        blk = C7_PE[0]
        while blk < C7_PE[1]:
            nr = 16 if C7_PE[1] - blk >= 16 else 8
            pc = ps2.tile([128, 1024], F32, tag="ps2")
            for t in range(49):
                di, dj = TAPS7[t]
                for h in range(nr // 8):
                    mv = vpad[:, 3 + blk + h * 8 + di:3 + blk + h * 8 + di + 8,
                              3 + dj:67 + dj]
                    nc.tensor.matmul(pc[:, h * 512:(h + 1) * 512], diag7_sb[:, t, :],
                                     mv, start=(t == 0), stop=(t == 48))
            nc.scalar.activation(out=xc[:, blk:blk + nr, :],
                                 in_=pc[:, 0:nr * 64],
                                 func=Act.Identity, bias=c7b_sb, scale=1.0)
            blk += nr

        # ---- xdown partial -> AllReduce -> xp_rep [128, T] bf16 (4x replicated) ----
        xp_part = big.tile([DS, T], BF16, tag="hs1")
        xcf = xc.rearrange("p a b -> p (a b)")
        for c4 in range(4):
            pxp = ps2.tile([128, 1024], F32, tag="ps2")
            for h in range(2):
                tb = c4 * 2 + h
                nc.tensor.matmul(pxp[0:DS, h * 512:(h + 1) * 512], xdT_sb,
                                 xcf[:, tb * 512:(tb + 1) * 512],
                                 start=True, stop=True)
            nc.scalar.copy(out=xp_part[:, c4 * 1024:(c4 + 1) * 1024],
                           in_=pxp[0:DS, :])
        ar_in = dram.tile([DS, T], BF16)
        ar_out = dram.tile([DS, T], BF16)
        nc.sync.dma_start(out=ar_in[:], in_=xp_part[:])
        nc.gpsimd.collective_compute(
            "AllReduce", Alu.add, replica_groups=PAIRS,
            ins=[ar_in.opt()], outs=[ar_out.opt()])
        # ---- scan: xe/xo materializations ----
        xe0 = big.tile([128, HW, 32], BF16, tag="xe0")
        nc.scalar.copy(out=xe0, in_=xc[:, :, 0::2])
        xo0 = big.tile([128, HW, 32], BF16, tag="xo0")
        nc.scalar.copy(out=xo0, in_=xc[:, :, 1::2])
        xcT = xc.rearrange("p h w -> p w h")
        xe1 = big.tile([128, HW, 32], BF16, tag="xe1")
        nc.scalar.copy(out=xe1, in_=xcT[:, :, 0::2])
        xo1 = big.tile([128, HW, 32], BF16, tag="xo1")
        nc.scalar.copy(out=xo1, in_=xcT[:, :, 1::2])
        xp_rep = big.tile([128, T], BF16, tag="accd", name="xprep")
        for f in range(4):
            nc.sync.dma_start(out=xp_rep[32 * f:32 * f + 16, :], in_=ar_out[:])
        xp3 = xp_rep.rearrange("p (h w) -> p h w", w=HW)

        xe = [xe0, xe1, xo0[:, :, ::-1], xo1[:, :, ::-1]]
        xo = [xo0, xo1, xe0[:, :, ::-1], xe1[:, :, ::-1]]

        Me = big.tile([128, HW, 32], BF16, tag="Me")
        Mo = big.tile([128, HW, 32], BF16, tag="Mo")

        GL, GM, GR, LQ, UQ, DQ = range(6)
        for f in range(4):
            fs = slice(32 * f, 32 * f + 16)
            tp = (32 * f, 0)
            sig = []
            for q in (GL, GM, GR):
                sg = mmo.tile([128, HW, 32], BF16, tag=f"sg{q}")
                for c2 in range(2):
                    pg = ps2.tile([128, 1024], F32, tag="ps2")
                    for h in range(2):
                        r = slice(c2 * 32 + h * 16, c2 * 32 + h * 16 + 16)
                        nc.tensor.matmul(pg[:, h * 512:(h + 1) * 512],
                                         gw_sb[fs, q, :], xp3[fs, r, 1::2],
                                         start=True, stop=True, tile_position=tp)
                    nc.scalar.activation(
                        out=sg[:, c2 * 32:c2 * 32 + 32, :],
                        in_=pg, func=Act.Sigmoid)
                sig.append(sg)
            sgl, sgm, sgr = sig
            par = {}
            for q in (LQ, UQ, DQ):
                qe = lud.tile([128, HW, 32], BF16, tag=f"q{q}e")
                qo = lud.tile([128, HW, 32], BF16, tag=f"q{q}o")
                for c4 in range(4):
                    pq = ps2.tile([128, 1024], F32, tag="ps2")
                    for h in range(2):
                        r = slice(c4 * 16 + h * 8, c4 * 16 + h * 8 + 8)
                        nc.tensor.matmul(pq[:, h * 512:(h + 1) * 512],
                                         gw_sb[fs, q, :], xp3[fs, r, :],
                                         start=True, stop=True, tile_position=tp)
                    p3 = pq.rearrange("p (a b) -> p a b", b=HW)
                    r16 = slice(c4 * 16, c4 * 16 + 16)
                    nc.scalar.copy(out=qe[:, r16, :], in_=p3[:, :, 0::2])
                    nc.scalar.copy(out=qo[:, r16, :], in_=p3[:, :, 1::2])
                par[q] = (qe, qo)
            l_e, l_o = par[LQ]
            u_e, u_o = par[UQ]
            d_e, d_o = par[DQ]

            # s-chain on Pool (starts as soon as sigmas land)
            s = tr.tile([128, HW, 32], F32, tag="s")
            nc.gpsimd.tensor_add(out=s, in0=sgl, in1=sgm)
            nc.gpsimd.tensor_add(out=s, in0=s, in1=sgr)
            nc.gpsimd.tensor_sub(out=s[:, 0, :], in0=s[:, 0, :], in1=sgl[:, 0, :])
            nc.gpsimd.tensor_sub(out=s[:, HW - 1, :], in0=s[:, HW - 1, :],
                                 in1=sgr[:, HW - 1, :])

            P = tr.tile([128, HW, 32], BF16, tag="P")
            nc.vector.tensor_mul(out=P, in0=l_e, in1=xe[f])
            TA = tr.tile([128, HW, 32], BF16, tag="TA")
            TB = tr.tile([128, HW, 32], BF16, tag="TB")
            TC = tr.tile([128, HW, 32], BF16, tag="TC")
            # even: Me (+)= (u_e*l_e + d_e) * xe
            nc.vector.tensor_mul(out=TA, in0=u_e, in1=l_e)
            nc.vector.tensor_add(out=TA, in0=TA, in1=d_e)
            if f == 0:
                nc.vector.tensor_mul(out=Me, in0=TA, in1=xe[f])
            else:
                nc.vector.tensor_mul(out=TB, in0=TA, in1=xe[f])
                nc.gpsimd.tensor_add(out=Me, in0=Me, in1=TB)
            # odd part 1: Mo (+)= (u_o*l_o + d_o) * xo
            nc.vector.tensor_mul(out=TA, in0=u_o, in1=l_o)
            nc.vector.tensor_add(out=TA, in0=TA, in1=d_o)
            if f == 0:
                nc.vector.tensor_mul(out=Mo, in0=TA, in1=xo[f])
            else:
                nc.vector.tensor_mul(out=TB, in0=TA, in1=xo[f])
                nc.vector.tensor_add(out=Mo, in0=Mo, in1=TB)
            nc.vector.tensor_scalar_max(out=s, in0=s, scalar1=1e-7)
            nc.vector.reciprocal_approx_fast(out=s, in_=s)
            nc.vector.tensor_mul(out=TA, in0=s, in1=u_o)
            # odd part 2: Mo += (u_o/s) * (sgm*P + sgl*P_up + sgr*P_dn)
            nc.vector.tensor_mul(out=TC, in0=sgm, in1=P)
            nc.vector.tensor_mul(out=TB[:, 1:, :], in0=sgl[:, 1:, :],
                                 in1=P[:, :HW - 1, :])
            nc.vector.tensor_add(out=TC[:, 1:, :], in0=TC[:, 1:, :],
                                 in1=TB[:, 1:, :])
            nc.vector.tensor_mul(out=TB[:, :HW - 1, :], in0=sgr[:, :HW - 1, :],
                                 in1=P[:, 1:, :])
            nc.vector.tensor_add(out=TC[:, :HW - 1, :], in0=TC[:, :HW - 1, :],
                                 in1=TB[:, :HW - 1, :])
            nc.vector.tensor_mul(out=TB, in0=TA, in1=TC)
            nc.vector.tensor_add(out=Mo, in0=Mo, in1=TB)

        # ---- exchange merged: AllGather pairs ----
        mg_in = dram.tile([128, 2, HW, 32], BF16)
        mg_out = dram.tile([2, 128, 2, HW, 32], BF16)
        nc.sync.dma_start(out=mg_in[:, 0], in_=Me[:])
        nc.sync.dma_start(out=mg_in[:, 1], in_=Mo[:])
        nc.gpsimd.collective_compute(
            "AllGather", Alu.bypass, replica_groups=PAIRS,
            ins=[mg_in.opt()], outs=[mg_out.opt()])
        mgF = big.tile([128, 2, 2, HW, 32], BF16, tag="hT")
        nc.sync.dma_start(out=mgF[:, 0], in_=mg_out[0])
        nc.sync.dma_start(out=mgF[:, 1], in_=mg_out[1])

        # ---- outconv (my 128 out-ch) -> y_pad [128, 66, 66] bf16 ----
        y_pad = big.tile([128, 66, 66], BF16, tag="vpad", name="ypad")
        nc.gpsimd.memset(y_pad, 0.0)
        for p in range(2):
            for c2 in range(2):
                py = ps2.tile([128, 1024], F32, tag="ps2")
                for h in range(2):
                    r = slice(c2 * 32 + h * 16, c2 * 32 + h * 16 + 16)
                    for kb in range(2):
                        nc.tensor.matmul(py[:, h * 512:(h + 1) * 512],
                                         ocT_sb[:, kb, :], mgF[:, kb, p, r, :],
                                         start=(kb == 0), stop=(kb == 1))
                nc.scalar.copy(
                    out=y_pad[:, 1 + c2 * 32:1 + c2 * 32 + 32, 1 + p:65 + p:2],
                    in_=py)

        # ---- dwconv3 (rows split) -> relu^2 -> y2 [128, T] bf16 ----
        y2 = big.tile([128, T], BF16, tag="xc")
        y23 = y2.rearrange("p (h w) -> p h w", w=HW)
        r0, r1 = D3_DVE
        a3 = big.tile([128, 32, HW], F32, tag="accd", name="acc3")[:, 0:r1 - r0, :]
        nc.vector.tensor_scalar_mul(out=a3, in0=y_pad[:, 1 + r0:1 + r1, 1:65],
                                    scalar1=k3_sb[:, 0:1])
        for t, (di, dj) in enumerate(TAPS3):
            if t == 0:
                continue
            srcv = y_pad[:, 1 + r0 + di:1 + r1 + di, 1 + dj:65 + dj]
            nc.vector.scalar_tensor_tensor(out=a3, in0=srcv,
                                           scalar=k3_sb[:, t:t + 1], in1=a3,
                                           op0=Alu.mult, op1=Alu.add)
        yr = rl.tile([128, r1 - r0, HW], BF16, tag="yr0")
        nc.vector.tensor_scalar_max(out=yr, in0=a3, scalar1=0.0)
        nc.scalar.square(out=y23[:, r0:r1, :], in_=yr)
        blk = D3_PE[0]
        while blk < D3_PE[1]:
            nr = 16 if D3_PE[1] - blk >= 16 else 8
            p3 = ps2.tile([128, 1024], F32, tag="ps2")
            for t in range(9):
                di, dj = TAPS3[t]
                for h in range(nr // 8):
                    mv = y_pad[:, 1 + blk + h * 8 + di:1 + blk + h * 8 + di + 8,
                               1 + dj:65 + dj]
                    nc.tensor.matmul(p3[:, h * 512:(h + 1) * 512], diag3_sb[:, t, :],
                                     mv, start=(t == 0), stop=(t == 8))
            yr = rl.tile([128, 16, HW], BF16, tag="yrpe")
            nc.vector.tensor_scalar_max(
                out=yr[:, 0:nr, :],
                in0=p3[:, 0:nr * 64].rearrange("p (a b) -> p a b", b=HW),
                scalar1=0.0)
            nc.scalar.square(out=y23[:, blk:blk + nr, :], in_=yr[:, 0:nr, :])
            blk += nr

        # ---- exchange y2: AllGather pairs ----
        y2_in = dram.tile([128, T], BF16)
        y2_out = dram.tile([2, 128, T], BF16)
        nc.sync.dma_start(out=y2_in[:], in_=y2[:])
        nc.gpsimd.collective_compute(
            "AllGather", Alu.bypass, replica_groups=PAIRS,
            ins=[y2_in.opt()], outs=[y2_out.opt()])
        y2F = big.tile([128, 2, T], BF16, tag="hs1")
        nc.sync.dma_start(out=y2F[:, 0], in_=y2_out[0])
        nc.sync.dma_start(out=y2F[:, 1], in_=y2_out[1])

        # ---- outproj (my 128 out cols) -> out [128, T] ----
        out_sb = big.tile([128, T], F32, tag="hs0")
        for c4 in range(4):
            po = ps2.tile([128, 1024], F32, tag="ps2")
            for h in range(2):
                tb = c4 * 2 + h
                for kb in range(2):
                    nc.tensor.matmul(po[:, h * 512:(h + 1) * 512], opT_sb[:, kb, :],
                                     y2F[:, kb, tb * 512:(tb + 1) * 512],
                                     start=(kb == 0), stop=(kb == 1))
            nc.scalar.copy(out=out_sb[:, c4 * 1024:(c4 + 1) * 1024], in_=po)
        nc.sync.dma_start(out=out_d[:, :], in_=out_sb)

    nc.compile()
    return nc


_CACHE = {}


def kernel(**inputs):
    if "nc" not in _CACHE:
        _CACHE["nc"] = build_program()
    nc = _CACHE["nc"]
    in_maps = host_prep(inputs)
    res = run_bass_kernel_spmd(nc, in_maps, list(range(NCORES)))
    outs = []
    for b in range(B):
        o0 = np.asarray(res.results[2 * b]["out"])
        o1 = np.asarray(res.results[2 * b + 1]["out"])
        outs.append(np.concatenate([o0.T, o1.T], axis=1))
    out = np.stack(outs, 0).astype(np.float32)
    shortcut = np.asarray(inputs["hidden_states"], dtype=np.float32)
    return out, shortcut

